# revision 5
# baseline (speedup 1.0000x reference)
"""MultiHeadAttention Trainium2 kernel, SPMD over 8 NeuronCores.

Problem: T=1024, B=4, E=1024, H=16 heads (head_dim 64), fp32.
  q = (query @ wq.T + bq) * Dh^-0.5 ; k, v likewise (unscaled)
  S = q @ k.T + attn_bias ; P = softmax(S) ; attn = (P @ v) @ wo.T + bo
  returns (attn [T,B,E], head_weights [H,B,T,T])

Sharding: batch*head parallel. Core c owns batch b=c//2 and heads
h0=(c%2)*8 .. h0+8 (8 consecutive heads = 512 feature columns).
q/k/v projections column-sharded by head, attention fully local per
head, out_proj row-sharded; the 2-way partial-sum per batch plus bo is
done on host (cheap: 8 x 4MB adds).

Device layout (per core):
  - host supplies query_b^T [E,T] so the contraction dim E lands on
    partitions; projections produce q^T/k^T [512,T] and v [T,512] (bf16).
  - scores S[t,s] accumulate in PSUM: QK^T matmul + identity-matmul that
    adds the attn_bias tile.
  - ACT does exp(S) PSUM->SBUF with fused row sums (accum_out); DVE
    reciprocal + per-partition tensor_scalar gives normalized P (f32)
    which is DMA'd straight out as head_weights.
  - P cast to bf16, PE-transposed per 128x128 block into a bf16 PSUM
    tile, evicted into P^T [s,t]; PV matmul (bf16) computes out^T per
    head pair; out_proj (bf16) computes attn_partial^T [E,T].
"""

import sys

if "/opt/trn_rl_repo" not in sys.path:
    sys.path.insert(0, "/opt/trn_rl_repo")

import numpy as np
import ml_dtypes

import concourse.bass as bass
import concourse.mybir as mybir
import concourse.tile as tile
from concourse import bacc
from concourse import bass_utils
from concourse.masks import make_identity

f32 = mybir.dt.float32
f32r = mybir.dt.float32r
bf16 = mybir.dt.bfloat16

# dtype of the QK^T score path ("f32r" for near-fp32 precision, "bf16" fallback)
QK_DT = f32r
# dtype of the bias-add identity matmul rhs path
BIAS_DT = f32r

P = 128
T = 1024
E = 1024
F = 512  # features per core (8 heads x 64)
DH = 64
HPC = 8  # heads per core
NT = T // P  # 8 t-tiles
NF = F // P  # 4 feature tiles
NE = E // P  # 8 embed tiles
NCH = T // 512  # 2 free-dim chunks of 512

N_CORES = 8
NUM_HEADS = 16
B = 4
SCALING = DH ** -0.5


def emit(tc):
    nc = tc.nc

    qT_d = nc.dram_tensor("qT", [E, T], f32r, kind="ExternalInput").ap()
    biasd_d = nc.dram_tensor("biasd", [HPC * T, T], BIAS_DT, kind="ExternalInput").ap()
    wqT_d = nc.dram_tensor("wqT", [E, F], f32r, kind="ExternalInput").ap()
    wkT_d = nc.dram_tensor("wkT", [E, F], f32r, kind="ExternalInput").ap()
    wvT_d = nc.dram_tensor("wvT", [E, F], f32r, kind="ExternalInput").ap()
    woT_d = nc.dram_tensor("woT", [F, E], bf16, kind="ExternalInput").ap()
    bq_d = nc.dram_tensor("bq", [F], f32, kind="ExternalInput").ap()
    bk_d = nc.dram_tensor("bk", [F], f32, kind="ExternalInput").ap()
    bv_d = nc.dram_tensor("bv", [1, F], f32r, kind="ExternalInput").ap()
    hw_d = nc.dram_tensor("hw", [HPC * T, T], f32, kind="ExternalOutput").ap()
    attnT_d = nc.dram_tensor("attnT", [E, T], f32, kind="ExternalOutput").ap()

    Exp = mybir.ActivationFunctionType.Exp
    Identity = mybir.ActivationFunctionType.Identity

    with tc.tile_pool(name="persist", bufs=1) as pers:
        identb = pers.tile([P, P], bf16, name="identb")
        make_identity(nc, identb)
        identf0 = pers.tile([P, P], f32, name="identf0")
        make_identity(nc, identf0)
        identf = pers.tile([P, P], BIAS_DT, name="identf")
        nc.vector.tensor_copy(identf, identf0)

        # persistent projection outputs
        qTs = [pers.tile([P, T], QK_DT, name=f"qTs{m}") for m in range(NF)]
        kTs = [pers.tile([P, T], QK_DT, name=f"kTs{m}") for m in range(NF)]
        v_sb = [pers.tile([P, F], bf16, name=f"vsb{t}") for t in range(NT)]
        outT = [pers.tile([P, T], bf16, name=f"outT{m}") for m in range(NF)]
        woT_sb = [pers.tile([P, E], bf16, name=f"woT{m}") for m in range(NF)]
        for m in range(NF):
            nc.sync.dma_start(woT_sb[m], woT_d[m * P : (m + 1) * P, :])

        bqs = pers.tile([P, NF], f32, name="bqs")
        bks = pers.tile([P, NF], f32, name="bks")
        nc.sync.dma_start(bqs, bq_d.rearrange("(m p) -> p m", p=P))
        nc.sync.dma_start(bks, bk_d.rearrange("(m p) -> p m", p=P))
        bvrow = pers.tile([1, F], f32r, name="bvrow")
        nc.sync.dma_start(bvrow, bv_d)
        onesrow0 = pers.tile([1, P], f32, name="onesrow0")
        nc.vector.memset(onesrow0, 1.0)
        onesrow = pers.tile([1, P], f32r, name="onesrow")
        nc.vector.tensor_copy(onesrow, onesrow0)

        # ---------------- Phase 1: projections ----------------
        with tc.tile_pool(name="p1sb", bufs=1) as p1, tc.tile_pool(
            name="p1ps", bufs=4, space="PSUM"
        ) as pp1:
            qin = [p1.tile([P, T], f32r, name=f"qin{k}") for k in range(NE)]
            for k in range(NE):
                nc.sync.dma_start(qin[k], qT_d[k * P : (k + 1) * P, :])
            wq_sb = [p1.tile([P, F], f32r, name=f"wq{k}") for k in range(NE)]
            wk_sb = [p1.tile([P, F], f32r, name=f"wk{k}") for k in range(NE)]
            wv_sb = [p1.tile([P, F], f32r, name=f"wv{k}") for k in range(NE)]
            for k in range(NE):
                nc.sync.dma_start(wq_sb[k], wqT_d[k * P : (k + 1) * P, :])
                nc.sync.dma_start(wk_sb[k], wkT_d[k * P : (k + 1) * P, :])
                nc.sync.dma_start(wv_sb[k], wvT_d[k * P : (k + 1) * P, :])

            # q^T and k^T: [F, T], bias per partition via ACT
            for w_sb, dst, bias_sb in ((wq_sb, qTs, bqs), (wk_sb, kTs, bks)):
                for m in range(NF):
                    for ch in range(NCH):
                        ps = pp1.tile([P, 512], f32, name="projps", tag="projps")
                        cs = slice(ch * 512, (ch + 1) * 512)
                        for k in range(NE):
                            nc.tensor.matmul(
                                ps,
                                lhsT=w_sb[k][:, m * P : (m + 1) * P],
                                rhs=qin[k][:, cs],
                                start=(k == 0),
                                stop=(k == NE - 1),
                            )
                        nc.scalar.activation(
                            out=dst[m][:, cs],
                            in_=ps,
                            func=Identity,
                            bias=bias_sb[:, m : m + 1],
                        )

            # v: [T, F] bf16 (+ bv broadcast row via K=1 matmul)
            for t in range(NT):
                ps = pp1.tile([P, 512], f32, name="vps", tag="projps")
                for k in range(NE):
                    nc.tensor.matmul(
                        ps,
                        lhsT=qin[k][:, t * P : (t + 1) * P],
                        rhs=wv_sb[k],
                        start=(k == 0),
                        stop=False,
                    )
                nc.tensor.matmul(ps, lhsT=onesrow, rhs=bvrow, start=False, stop=True)
                nc.scalar.copy(v_sb[t], ps)

        # ---------------- Phase 2: attention ----------------
        with tc.tile_pool(name="biasp", bufs=3) as biasp, tc.tile_pool(
            name="pf32p", bufs=3
        ) as pf32p, tc.tile_pool(name="pnp", bufs=3) as pnp, tc.tile_pool(
            name="pbp", bufs=3
        ) as pbp, tc.tile_pool(name="smallp", bufs=6) as smallp, tc.tile_pool(
            name="ptp", bufs=3
        ) as ptp, tc.tile_pool(
            name="sps", bufs=2, space="PSUM"
        ) as spsum, tc.tile_pool(name="tps", bufs=2, space="PSUM") as tpsum, tc.tile_pool(
            name="pvps", bufs=2, space="PSUM"
        ) as pvpsum:
            for hp in range(4):
                pts = []
                for hh in range(2):
                    h = 2 * hp + hh
                    base = 64 * hh
                    PT = ptp.tile([P, NT, T], bf16, name=f"PT{h}", tag="PT")
                    pts.append(PT)
                    for tt in range(NT):
                        row = (h * NT + tt) * P
                        bias_t = biasp.tile([P, T], BIAS_DT, name="bias_t", tag="bias")
                        nc.sync.dma_start(bias_t, biasd_d[row : row + P, :])
                        S_ps = spsum.tile([P, T], f32, name="S_ps", tag="S")
                        lq = qTs[hp][base : base + DH, tt * P : (tt + 1) * P]
                        for ch in range(NCH):
                            cs = slice(ch * 512, (ch + 1) * 512)
                            nc.tensor.matmul(
                                S_ps[:, cs],
                                lhsT=lq,
                                rhs=kTs[hp][base : base + DH, cs],
                                start=True,
                                stop=False,
                            )
                            nc.tensor.matmul(
                                S_ps[:, cs],
                                lhsT=identf,
                                rhs=bias_t[:, cs],
                                start=False,
                                stop=True,
                            )
                        Pf = pf32p.tile([P, T], f32, name="Pf", tag="Pf")
                        sums = smallp.tile([P, 1], f32, name="sums", tag="sums")
                        nc.scalar.activation(out=Pf, in_=S_ps, func=Exp, accum_out=sums)
                        inv = smallp.tile([P, 1], f32, name="inv", tag="inv")
                        nc.vector.reciprocal(inv, sums)
                        Pn = pnp.tile([P, T], f32, name="Pn", tag="Pn")
                        nc.vector.tensor_scalar_mul(Pn, Pf, inv)
                        nc.sync.dma_start(hw_d[row : row + P, :], Pn)
                        Pb = pbp.tile([P, T], bf16, name="Pb", tag="Pb")
                        nc.vector.tensor_copy(out=Pb, in_=Pn)
                        tp_ps = tpsum.tile([P, T], bf16, name="tp_ps", tag="tp")
                        for st in range(NT):
                            nc.tensor.transpose(
                                tp_ps[:, st * P : (st + 1) * P],
                                Pb[:, st * P : (st + 1) * P],
                                identb,
                            )
                        nc.vector.tensor_copy(
                            out=PT[:, :, tt * P : (tt + 1) * P],
                            in_=tp_ps.rearrange("p (st c) -> p st c", st=NT),
                        )
                # PV for the pair -> outT[hp]
                for ch in range(NCH):
                    cs = slice(ch * 512, (ch + 1) * 512)
                    pv_ps = pvpsum.tile([P, 512], f32, name="pv_ps", tag="pv")
                    for hh in range(2):
                        h = 2 * hp + hh
                        base = 64 * hh
                        for st in range(NT):
                            nc.tensor.matmul(
                                pv_ps[base : base + DH, :],
                                lhsT=v_sb[st][:, h * DH : (h + 1) * DH],
                                rhs=pts[hh][:, st, cs],
                                start=(st == 0),
                                stop=(st == NT - 1),
                            )
                    nc.scalar.copy(outT[hp][:, cs], pv_ps)

        # ---------------- Phase 3: out_proj ----------------
        with tc.tile_pool(name="p3sb", bufs=3) as p3, tc.tile_pool(
            name="p3ps", bufs=3, space="PSUM"
        ) as pp3:
            for mt in range(NE):
                for ch in range(NCH):
                    cs = slice(ch * 512, (ch + 1) * 512)
                    fo_ps = pp3.tile([P, 512], f32, name="fo_ps", tag="fo")
                    for kt in range(NF):
                        nc.tensor.matmul(
                            fo_ps,
                            lhsT=woT_sb[kt][:, mt * P : (mt + 1) * P],
                            rhs=outT[kt][:, cs],
                            start=(kt == 0),
                            stop=(kt == NF - 1),
                        )
                    stg = p3.tile([P, 512], f32, name="stg", tag="stg")
                    nc.vector.tensor_copy(stg, fo_ps)
                    nc.sync.dma_start(attnT_d[mt * P : (mt + 1) * P, cs], stg)


_CACHE = {}


def build():
    if "nc" in _CACHE:
        return _CACHE["nc"]
    nc = bacc.Bacc(
        "TRN2",
        target_bir_lowering=False,
        debug=False,
        enable_asserts=False,
        num_devices=N_CORES,
    )
    with tile.TileContext(nc, trace_sim=False) as tc:
        emit(tc)
    nc.compile()
    _CACHE["nc"] = nc
    return nc


def make_in_maps(query, attn_bias, wq, bq, wk, bk, wv, bv, wo, bo):
    query = np.asarray(query, dtype=np.float32)
    attn_bias = np.asarray(attn_bias, dtype=np.float32)
    wq = np.asarray(wq, dtype=np.float32)
    wk = np.asarray(wk, dtype=np.float32)
    wv = np.asarray(wv, dtype=np.float32)
    wo = np.asarray(wo, dtype=np.float32)
    bq = np.asarray(bq, dtype=np.float32)
    bk = np.asarray(bk, dtype=np.float32)
    bv = np.asarray(bv, dtype=np.float32)

    in_maps = []
    for c in range(N_CORES):
        b = c // 2
        h0 = (c % 2) * HPC
        fsl = slice(h0 * DH, h0 * DH + F)
        qT = np.ascontiguousarray(query[:, b, :].T)
        in_maps.append(
            {
                "qT": qT,
                "biasd": np.ascontiguousarray(
                    attn_bias[c * HPC : (c + 1) * HPC].reshape(HPC * T, T)
                ),
                "wqT": np.ascontiguousarray(wq[fsl, :].T) * np.float32(SCALING),
                "wkT": np.ascontiguousarray(wk[fsl, :].T),
                "wvT": np.ascontiguousarray(wv[fsl, :].T),
                "woT": np.ascontiguousarray(wo[:, fsl].T).astype(ml_dtypes.bfloat16),
                "bq": np.ascontiguousarray(bq[fsl]) * np.float32(SCALING),
                "bk": np.ascontiguousarray(bk[fsl]),
                "bv": np.ascontiguousarray(bv[fsl]).reshape(1, F),
            }
        )
    return in_maps


def gather(results, bo):
    bo = np.asarray(bo, dtype=np.float32)
    head_weights = np.empty((NUM_HEADS, B, T, T), dtype=np.float32)
    attn = np.empty((T, B, E), dtype=np.float32)
    for c in range(N_CORES):
        b = c // 2
        h0 = (c % 2) * HPC
        hw = results[c]["hw"].reshape(HPC, T, T)
        head_weights[h0 : h0 + HPC, b] = hw
    for b in range(B):
        s = results[2 * b]["attnT"] + results[2 * b + 1]["attnT"]
        attn[:, b, :] = s.T + bo[None, :]
    return attn, head_weights


def kernel(query, attn_bias, wq, bq, wk, bk, wv, bv, wo, bo, _trace=False):
    nc = build()
    in_maps = make_in_maps(query, attn_bias, wq, bq, wk, bk, wv, bv, wo, bo)
    res = bass_utils.run_bass_kernel_spmd(
        nc, in_maps, core_ids=list(range(N_CORES)), trace=_trace
    )
    out = gather(res.results, bo)
    if _trace:
        _CACHE["last_result"] = res
    return out
